# revision 36
# baseline (speedup 1.0000x reference)
"""CvtSelfAttention TRN2 Bass kernel.

Strategy (8 NeuronCores, batch data-parallel, 4 batch elems per core):

The PE-column floor of this problem (fp32r/bf16 matmuls at 1 col/cycle) is
~110.6k columns per batch elem for projections + attention; the depthwise
convs would add another ~83k columns as diagonal matmuls, so they are moved
OFF the PE onto the DVE and Pool engines as bf16 multiply-add pair chains
(tensor_scalar -> tmp at 4x DVE rate, tensor_tensor add in place), leaving
the PE as the sole ~45us/batch bottleneck.

Device (per batch elem):
  1. Depthwise 3x3 convs in bf16 on DVE (q + v[ct<3]) and Pool (k + v[ct>=3]):
     9 taps, first tap fused with the BN shift via tensor_scalar, remaining
     8 taps as (ts mul -> tmp, tt add) pairs. Stride-2 k/v taps read an
     even/odd-plane repack so all views are contiguous (4x DVE mode).
     Emission is software-pipelined: conv(b+1) is emitted BEFORE the
     attention block of batch b so the DVE/Pool FIFOs never head-of-line
     block the next batch's conv behind attention evictions.
  2. QKV linear projections as bf16 matmuls (conv outputs x bf16 W^T tiles),
     fp32 PSUM, evicted with the torch-Linear bias by ACT (q/k) into f32r
     tiles, or scatter-copied by Pool into the V_aug layout (v; bias b_v is
     folded on the host into the final merge: + b_v * den).
  3. Per-head attention in f32r exactly as the baseline: scoresT = K^T Q
     (head pairs on PE row groups 0/64), exp on ACT (scores ~N(0,.1), no max
     subtraction), ctxT_aug = V_aug^T @ expT with a ones column giving the
     softmax denominator row. kproj/vproj of batch b+1 are interleaved as PE
     filler between heads: the ACT exp stream (2.1us/head) is slower than the
     PE scores+ctx stream (1.7us/head), and the filler absorbs the gap.
  4. cls-KEY scores for all 12 heads in one stuffed block-diagonal matmul.
Host (numpy, exact fp32; ~0.1% of FLOPs): input layout prep (bf16 padded /
even-odd planes, BN fold, W^T tiles), cls-query attention row, merge of the
cls-key term and b_v into ctx, final [B, L, E] assembly.
"""
import os
import sys
import numpy as np
import ml_dtypes

for _p in ("/opt/trn_rl_repo", "/root/.axon_site/_ro/trn_rl_repo"):
    if os.path.isdir(_p) and _p not in sys.path:
        sys.path.append(_p)

import concourse.bass as bass
import concourse.bacc as bacc
import concourse.tile as tile
from concourse import mybir
from concourse.bass_utils import run_bass_kernel_spmd

EMBED = 768
HEADS = 12
D = 64
EPS = 1e-5
NCORES = 8
B_TOTAL = 32
NB = B_TOTAL // NCORES          # batch elems per core
CT = EMBED // 128               # 6 c-tiles
NPIX = 1024                     # stride-1 conv output pixels (i-dim on device)
NKV = 256                       # stride-2 conv output pixels (j-dim on device)
SM_SCALE = float(EMBED) ** -0.5

F32 = mybir.dt.float32
F32R = mybir.dt.float32r
BF16 = mybir.dt.bfloat16
AO = mybir.AluOpType
BF = ml_dtypes.bfloat16

# set by kernel() for test harnesses to inspect
last_results = None


def _build_program(repeat: int = 1):
    nc = bacc.Bacc(None, target_bir_lowering=False, debug=False)

    host_conv = bool(os.environ.get("CONV_HOST"))
    kv_host = bool(os.environ.get("CONV_KV_HOST"))
    # ---- DRAM I/O (per core) ----
    if host_conv:
        cq_d = nc.dram_tensor("conv_q", [NB, 128, CT, NPIX], BF16,
                              kind="ExternalInput")
    else:
        xq_d = nc.dram_tensor("xq", [NB, CT, 128, 1156], BF16,
                              kind="ExternalInput")
    if host_conv or kv_host:
        ck_d = nc.dram_tensor("conv_k", [NB, 128, CT, NKV], BF16,
                              kind="ExternalInput")
        cv_d = nc.dram_tensor("conv_v", [NB, 128, CT, NKV], BF16,
                              kind="ExternalInput")
    elif not host_conv:
        xkv_d = nc.dram_tensor("xkv", [NB, CT, 128, 1156], BF16,
                               kind="ExternalInput")
    wcol = nc.dram_tensor("wcol", [128, 3 * CT * 9], F32, kind="ExternalInput")
    w_t = nc.dram_tensor("w_t", [3, CT, 128, EMBED], BF16, kind="ExternalInput")
    shifts = nc.dram_tensor("shifts", [128, 3 * CT], F32, kind="ExternalInput")
    biases = nc.dram_tensor("biases", [128, 2 * CT], F32, kind="ExternalInput")
    vtpl = nc.dram_tensor("vtpl", [128, HEADS * 128], F32R, kind="ExternalInput")
    kcls = nc.dram_tensor("kcls", [NB, 128, CT * HEADS], F32R, kind="ExternalInput")

    dbg_conv = None
    if os.environ.get("DBG_CONV"):
        dbg_conv = nc.dram_tensor("dbg_conv", [NB, 128, CT, NPIX + 2 * NKV], BF16,
                                  kind="ExternalOutput")
    ctxu = nc.dram_tensor("ctxu", [NB, HEADS, D + 1, NPIX], F32, kind="ExternalOutput")
    ecls = nc.dram_tensor("ecls", [NB, HEADS, NPIX], F32, kind="ExternalOutput")
    kt_out = nc.dram_tensor("kt_out", [NB, 128, CT, NKV], F32R, kind="ExternalOutput")
    v_out = nc.dram_tensor("v_out", [NB, 128, 2, HEADS * D], F32R,
                           kind="ExternalOutput")

    with tile.TileContext(nc) as tc:
        import contextlib
        with contextlib.ExitStack() as ctx:
            def _bufs(name, dflt):
                return int(os.environ.get(f"BUFS_{name}", dflt))

            consts = ctx.enter_context(tc.tile_pool(name="consts", bufs=1))
            xqp = ctx.enter_context(tc.tile_pool(name="xqp", bufs=_bufs("XQ", 3)))
            xkp = ctx.enter_context(tc.tile_pool(name="xkp", bufs=_bufs("XK", 3)))
            convp = ctx.enter_context(tc.tile_pool(name="convp", bufs=_bufs("CONV", 2)))
            tmpp = ctx.enter_context(tc.tile_pool(name="tmpp", bufs=2))
            qtp = ctx.enter_context(tc.tile_pool(name="qtp", bufs=_bufs("QT", 1)))
            ktp = ctx.enter_context(tc.tile_pool(name="ktp", bufs=_bufs("KT", 2)))
            vap = ctx.enter_context(tc.tile_pool(name="vap", bufs=_bufs("VA", 2)))
            kclsp = ctx.enter_context(tc.tile_pool(name="kclsp", bufs=2))
            expp = ctx.enter_context(tc.tile_pool(name="expp", bufs=_bufs("EXP", 3)))
            stage = ctx.enter_context(tc.tile_pool(name="stage", bufs=_bufs("ST", 3)))
            eclsp = ctx.enter_context(tc.tile_pool(name="eclsp", bufs=2))
            bigps = ctx.enter_context(
                tc.tile_pool(name="bigps", bufs=_bufs("BIG", 2), space="PSUM"))
            ps10 = ctx.enter_context(
                tc.tile_pool(name="ps10", bufs=_bufs("PS10", 2), space="PSUM"))

            # ---- constants ----
            if not host_conv:
                wcol_sb = consts.tile([128, 3 * CT * 9], F32)
                nc.sync.dma_start(wcol_sb, wcol[:, :])
                wcol_v = wcol_sb.rearrange("p (c t k) -> p c t k", c=3, t=CT)
                shifts_sb = consts.tile([128, 3 * CT], F32)
                nc.sync.dma_start(shifts_sb, shifts[:, :])
                shifts_v = shifts_sb.rearrange("p (c t) -> p c t", c=3)
            bias_sb = consts.tile([128, 2 * CT], F32)
            nc.sync.dma_start(bias_sb, biases[:, :])
            bias_v = bias_sb.rearrange("p (c t) -> p c t", c=2)
            ws = []
            for cv in range(3):
                row = [consts.tile([128, EMBED], BF16, name=f"w{cv}_{i}")
                       for i in range(CT)]
                for kt in range(CT):
                    nc.sync.dma_start(row[kt], w_t[cv, kt])
                ws.append(row)
            wq, wk, wv = ws
            # persistent V_aug double buffer: ones column (softmax denominator
            # row) written once; per-batch vproj evicts only touch cols 0:D of
            # each head block, so the template survives reuse.
            vaugs = [vap.tile([128, 2, HEADS * 128], F32R, tag="vaug",
                              name=f"vaug{i}") for i in range(2)]
            for vg in vaugs:
                nc.sync.dma_start(vg[:, 0, :], vtpl[:, :])
                nc.sync.dma_start(vg[:, 1, :], vtpl[:, :])

            def conv_chain(eng, dst, w_sc, shift_sc, view, nfree):
                # dst += sum_t w_t * x_t  (+shift on tap 0), all bf16
                tmp = tmpp.tile([128, nfree], BF16,
                                tag=f"tmp_{eng is nc.vector}_{nfree}")
                tv = tmp.rearrange("p (a b) -> p a b", b=view(0).shape[-1])
                eng.tensor_scalar(out=dst, in0=view(0), scalar1=w_sc(0),
                                  scalar2=shift_sc, op0=AO.mult, op1=AO.add)
                for tap in range(1, 9):
                    eng.tensor_scalar_mul(tv, view(tap), w_sc(tap))
                    eng.tensor_tensor(dst, tv, dst, AO.add)

            def emit_conv(b):
                """conv for batch b: host-computed DMA load, or DVE/Pool
                bf16 pair chains."""
                convq = convp.tile([128, CT, NPIX], BF16, tag="convq")
                convk = convp.tile([128, CT, NKV], BF16, tag="convk")
                convv = convp.tile([128, CT, NKV], BF16, tag="convv")
                if host_conv:
                    nc.sync.dma_start(convq, cq_d[b])
                if host_conv or kv_host:
                    nc.sync.dma_start(convk, ck_d[b])
                    nc.sync.dma_start(convv, cv_d[b])
                    if host_conv:
                        return convq, convk, convv
                for ct in range(CT):
                    xqt = xqp.tile([128, 1164], BF16, tag="xq")
                    nc.sync.dma_start(xqt[:, 0:1156], xq_d[b, ct])
                    if not kv_host:
                        xkt = xkp.tile([128, 1164], BF16, tag="xk")
                        nc.sync.dma_start(xkt[:, 0:1156], xkv_d[b, ct])

                    def qview(tap):
                        dh, dw = tap // 3, tap % 3
                        base = dh * 34 + dw
                        return xqt[:, base:base + 32 * 34].rearrange(
                            "p (r c) -> p r c", c=34)[:, :, 0:32]

                    def kview(tap):
                        dh, dw = tap // 3, tap % 3
                        base = (dw & 1) * 34 * 17 + dh * 17 + dw // 2
                        return xkt[:, base:base + 16 * 34].rearrange(
                            "p (r c) -> p r c", c=34)[:, :, 0:16]

                    conv_chain(
                        nc.vector,
                        convq[:, ct, :].rearrange("p (a b) -> p a b", b=32),
                        lambda t: wcol_v[:, 0, ct, t:t + 1],
                        shifts_v[:, 0, ct:ct + 1], qview, NPIX)
                    if not kv_host:
                        kv_eng = (nc.vector if os.environ.get("NO_POOL")
                                  else nc.gpsimd)
                        conv_chain(
                            kv_eng,
                            convk[:, ct, :].rearrange("p (a b) -> p a b", b=16),
                            lambda t: wcol_v[:, 1, ct, t:t + 1],
                            shifts_v[:, 1, ct:ct + 1], kview, NKV)
                        conv_chain(
                            kv_eng,
                            convv[:, ct, :].rearrange("p (a b) -> p a b", b=16),
                            lambda t: wcol_v[:, 2, ct, t:t + 1],
                            shifts_v[:, 2, ct:ct + 1], kview, NKV)
                return convq, convk, convv

            def emit_kproj_chunk(convk, ktile, et):
                pk = bigps.tile([128, NPIX], F32, tag="big")
                for kt in range(CT):
                    nc.tensor.matmul(
                        pk[:, 0:NKV], wk[kt][:, et * 128:(et + 1) * 128],
                        convk[:, kt, :],
                        start=(kt == 0), stop=(kt == CT - 1))
                nc.scalar.activation(
                    ktile[:, et, :], pk[:, 0:NKV],
                    mybir.ActivationFunctionType.Identity,
                    bias=bias_v[:, 1, et:et + 1])

            def emit_vproj_chunk(convv, vaug, jt):
                pv = bigps.tile([128, NPIX], F32, tag="big")
                for ch, (e0, en) in enumerate([(0, 512), (512, 256)]):
                    for kt in range(CT):
                        nc.tensor.matmul(
                            pv[:, e0:e0 + en],
                            convv[:, kt, jt * 128:(jt + 1) * 128],
                            wv[kt][:, e0:e0 + en],
                            start=(kt == 0), stop=(kt == CT - 1))
                # scatter heads into the 128-strided V_aug layout (ACT; Pool
                # cannot read PSUM)
                dstv = vaug[:, jt, :].rearrange("p (h d) -> p h d", d=128)[:, :, 0:D]
                nc.scalar.copy(dstv,
                               pv[:, 0:EMBED].rearrange("p (h d) -> p h d", d=D))

            # =================== program ===================
            no_pipe = bool(os.environ.get("NO_PIPELINE"))
            # prologue: conv for batch 0
            conv_next = None if no_pipe else emit_conv(0)
            kv_cur = None          # (ktile, vaug) produced by fillers for b

            rep_ctx = tc.For_i(0, repeat, 1) if repeat > 1 else None
            if rep_ctx is not None:
                rep_ctx.__enter__()
            for b in range(NB):
                if no_pipe:
                    convq, convk, convv = emit_conv(b)
                else:
                    convq, convk, convv = conv_next
                if dbg_conv is not None:
                    nc.scalar.dma_start(dbg_conv[b][:, :, 0:NPIX], convq)
                    nc.scalar.dma_start(dbg_conv[b][:, :, NPIX:NPIX + NKV], convk)
                    nc.scalar.dma_start(dbg_conv[b][:, :, NPIX + NKV:], convv)
                if not no_pipe:
                    # conv for next batch first: keeps DVE/Pool FIFOs ahead
                    conv_next = emit_conv((b + 1) % NB)
                if os.environ.get("CONV_ONLY"):
                    continue

                # ---- k/v projections for THIS batch if not already done
                # (batch 0 of each repeat iteration reuses the fillers from
                # the previous iteration's last batch; first time they must
                # be emitted inline) ----
                if kv_cur is None:
                    ktile = ktp.tile([128, CT, NKV], F32R, tag="ktile")
                    vaug = vaugs[b % 2]
                    for et in range(CT):
                        emit_kproj_chunk(convk, ktile, et)
                    for jt in range(2):
                        emit_vproj_chunk(convv, vaug, jt)
                    nc.scalar.dma_start(kt_out[b], ktile[:, :, :])
                    nc.scalar.dma_start(
                        v_out[b],
                        vaug.rearrange("p j (h d) -> p j h d", d=128)[:, :, :, 0:D])
                else:
                    ktile, vaug = kv_cur

                # ---- q projection ----
                qt = qtp.tile([128, CT, NPIX], F32R, tag="qt")
                for et in range(CT):
                    pq = bigps.tile([128, NPIX], F32, tag="big")
                    for ch in range(2):
                        for kt in range(CT):
                            nc.tensor.matmul(
                                pq[:, ch * 512:(ch + 1) * 512],
                                wq[kt][:, et * 128:(et + 1) * 128],
                                convq[:, kt, ch * 512:(ch + 1) * 512],
                                start=(kt == 0), stop=(kt == CT - 1))
                    nc.scalar.activation(
                        qt[:, et, :], pq, mybir.ActivationFunctionType.Identity,
                        bias=bias_v[:, 0, et:et + 1])

                # ---- cls-key scores for all heads ----
                kc = kclsp.tile([128, CT * HEADS], F32R)
                nc.sync.dma_start(kc, kcls[b])
                kc_v = kc.rearrange("p (t h) -> p t h", t=CT)
                pcls = bigps.tile([128, NPIX], F32, tag="big")
                for ch in range(2):
                    for kt in range(CT):
                        nc.tensor.matmul(
                            pcls[0:12, ch * 512:(ch + 1) * 512], kc_v[:, kt, :],
                            qt[:, kt, ch * 512:(ch + 1) * 512],
                            start=(kt == 0), stop=(kt == CT - 1))
                ec = eclsp.tile([12, NPIX], F32)
                nc.scalar.activation(ec, pcls[0:12, :],
                                     mybir.ActivationFunctionType.Exp,
                                     scale=SM_SCALE)
                nc.scalar.dma_start(ecls[b], ec)

                # ---- k/v projections for NEXT batch, as attention fillers.
                # Batch 0 of each repeat iteration is computed inline instead
                # (its reads are trace-bound to the inline tiles), so the last
                # batch runs without fillers. ----
                bn = (b + 1) % NB
                fillers = []
                if bn != 0 and not no_pipe and not os.environ.get("NO_FILLERS"):
                    nktile = ktp.tile([128, CT, NKV], F32R, tag="ktile")
                    nvaug = vaugs[bn % 2]
                    nconvk, nconvv = conv_next[1], conv_next[2]
                    for et in range(CT):
                        fillers.append(
                            lambda et=et: emit_kproj_chunk(nconvk, nktile, et))
                    for jt in range(2):
                        fillers.append(
                            lambda jt=jt: emit_vproj_chunk(nconvv, nvaug, jt))
                    fillers.append(
                        lambda: nc.scalar.dma_start(kt_out[bn], nktile[:, :, :]))
                    fillers.append(lambda: nc.scalar.dma_start(
                        v_out[bn],
                        nvaug.rearrange("p j (h d) -> p j h d", d=128)[:, :, :, 0:D]))
                    kv_cur = (nktile, nvaug)
                else:
                    kv_cur = None
                fit = iter(fillers)

                # ---- attention ----
                for g in range(CT):  # head pairs on PE row groups 0/64
                    for hh in range(2):
                        h = 2 * g + hh
                        p0, p1 = 64 * hh, 64 * hh + 64
                        et = [None, None]
                        for jt in range(2):
                            pst = ps10.tile([128, NPIX], F32, tag="ps10")
                            for ch in range(2):
                                nc.tensor.matmul(
                                    pst[:, ch * 512:(ch + 1) * 512],
                                    ktile[p0:p1, g, jt * 128:(jt + 1) * 128],
                                    qt[p0:p1, g, ch * 512:(ch + 1) * 512],
                                    start=True, stop=True)
                            ex = expp.tile([128, NPIX], F32R)
                            nc.scalar.activation(
                                ex, pst, mybir.ActivationFunctionType.Exp,
                                scale=SM_SCALE)
                            et[jt] = ex
                        for fn in (next(fit, None),):
                            if fn is not None:
                                fn()
                        pc = bigps.tile([128, NPIX], F32, tag="big")
                        for ch in range(2):
                            for jt in range(2):
                                nc.tensor.matmul(
                                    pc[:, ch * 512:(ch + 1) * 512],
                                    vaug[:, jt, h * 128:(h + 1) * 128],
                                    et[jt][:, ch * 512:(ch + 1) * 512],
                                    start=(jt == 0), stop=(jt == 1))
                        st = stage.tile([D + 1, NPIX], F32)
                        if h < 8:
                            nc.vector.tensor_copy(st, pc[0:D + 1, :])
                        else:
                            nc.scalar.copy(st, pc[0:D + 1, :])
                        nc.scalar.dma_start(ctxu[b, h], st)
                # drain unused fillers (12 heads, 12 fillers -> none left)
                for fn in fit:
                    fn()
            if rep_ctx is not None:
                rep_ctx.__exit__(None, None, None)

    nc.finalize()
    return nc


def _host_prep(inputs):
    x = np.ascontiguousarray(inputs["x"], dtype=np.float32)     # [B, 1025, 768]
    B = x.shape[0]
    prep = {}

    cw = {}
    shift = {}
    for p in ["q", "k", "v"]:
        g = np.asarray(inputs[f"bn_g_{p}"], np.float32)
        be = np.asarray(inputs[f"bn_b_{p}"], np.float32)
        m = np.asarray(inputs[f"bn_m_{p}"], np.float32)
        v = np.asarray(inputs[f"bn_v_{p}"], np.float32)
        s = g / np.sqrt(v + EPS)
        cw[p] = np.asarray(inputs[f"conv_w_{p}"], np.float32)[:, 0] * s[:, None, None]
        shift[p] = be - m * s

    # padded 34x34 CHW image, bf16
    hs = x[:, 1:, :].reshape(B, 32, 32, EMBED)
    xp = np.zeros((B, EMBED, 34, 34), np.float32)
    xp[:, :, 1:33, 1:33] = hs.transpose(0, 3, 1, 2)
    host_conv = bool(os.environ.get("CONV_HOST"))
    kv_host = bool(os.environ.get("CONV_KV_HOST"))
    host_projs = ([("conv_q", "q", 1), ("conv_k", "k", 2), ("conv_v", "v", 2)]
                  if host_conv else
                  ([("conv_k", "k", 2), ("conv_v", "v", 2)] if kv_host else []))
    for name, p, stride in host_projs:
        n = 32 // stride
        acc = np.broadcast_to(shift[p][None, :, None, None],
                              (B, EMBED, n, n)).copy()
        for dh in range(3):
            for dw in range(3):
                acc += cw[p][:, dh, dw][None, :, None, None] * \
                    xp[:, :, dh:dh + 32:stride, dw:dw + 32:stride]
        prep[name] = np.ascontiguousarray(
            acc.reshape(B, CT, 128, n * n).transpose(0, 2, 1, 3).astype(BF))
    if not host_conv:
        xp_bf = xp.astype(BF)
        prep["xq"] = np.ascontiguousarray(xp_bf.reshape(B, CT, 128, 1156))
        if not kv_host:
            # even/odd column planes for stride-2 convs: [B,CT,128,2,34,17]
            xkv = np.empty((B, EMBED, 2, 34, 17), BF)
            xkv[:, :, 0] = xp_bf[:, :, :, 0::2]
            xkv[:, :, 1] = xp_bf[:, :, :, 1::2]
            prep["xkv"] = np.ascontiguousarray(xkv.reshape(B, CT, 128, 1156))

    # wcol: [128, 3*CT*9] per-partition conv-tap weights
    wc = np.zeros((128, 3, CT, 9), np.float32)
    for ci, p in enumerate(["q", "k", "v"]):
        w9 = cw[p].reshape(EMBED, 9)                  # [c, tap]
        for ct in range(CT):
            wc[:, ci, ct, :] = w9[ct * 128:(ct + 1) * 128, :]
    prep["wcol"] = np.ascontiguousarray(wc.reshape(128, -1))

    # w_t: [3, 6, 128, 768] bf16: W^T split into k-tiles
    prep["w_t"] = np.ascontiguousarray(np.stack([
        np.asarray(inputs[f"W_{p}"], np.float32).T.reshape(CT, 128, EMBED)
        for p in ["q", "k", "v"]]).astype(BF))

    # shifts [128, 3*6], biases [128, 2*6]
    sh = np.stack([shift[p].reshape(CT, 128) for p in ["q", "k", "v"]])  # [3,6,128]
    prep["shifts"] = np.ascontiguousarray(sh.transpose(2, 0, 1).reshape(128, -1))
    bb = np.stack([np.asarray(inputs[f"b_{p}"], np.float32).reshape(CT, 128)
                   for p in ["q", "k"]])                                  # [2,6,128]
    prep["biases"] = np.ascontiguousarray(bb.transpose(2, 0, 1).reshape(128, -1))

    # host-exact cls projections
    cls = x[:, 0, :]                                               # [B, 768]
    Wq = np.asarray(inputs["W_q"], np.float32)
    Wk = np.asarray(inputs["W_k"], np.float32)
    Wv = np.asarray(inputs["W_v"], np.float32)
    prep["q_cls"] = cls @ Wq.T + np.asarray(inputs["b_q"], np.float32)
    k_cls = cls @ Wk.T + np.asarray(inputs["b_k"], np.float32)
    prep["k_cls"] = k_cls
    prep["v_cls"] = cls @ Wv.T + np.asarray(inputs["b_v"], np.float32)
    prep["b_v"] = np.asarray(inputs["b_v"], np.float32)

    vt = np.zeros((128, HEADS, 128), np.float32)
    vt[:, :, D] = 1.0
    prep["vtpl"] = np.ascontiguousarray(vt.reshape(128, HEADS * 128))

    # kcls stuffed block lhsT: [B, 128, 6*12]
    kc = np.zeros((B, CT, 128, HEADS), np.float32)
    crange = np.arange(EMBED)
    hofc = crange // D                                             # head of channel
    for b in range(B):
        kc[b, crange // 128, crange % 128, hofc] = k_cls[b]
    prep["kcls"] = np.ascontiguousarray(kc.transpose(0, 2, 1, 3).reshape(B, 128, -1))
    return prep


def _in_maps(prep):
    maps = []
    for c in range(NCORES):
        sl = slice(c * NB, (c + 1) * NB)
        m = {
            "wcol": prep["wcol"],
            "w_t": prep["w_t"],
            "shifts": prep["shifts"],
            "biases": prep["biases"],
            "vtpl": prep["vtpl"],
            "kcls": prep["kcls"][sl],
        }
        host_conv = bool(os.environ.get("CONV_HOST"))
        kv_host = bool(os.environ.get("CONV_KV_HOST"))
        if host_conv:
            m["conv_q"] = prep["conv_q"][sl]
        else:
            m["xq"] = prep["xq"][sl]
        if host_conv or kv_host:
            m["conv_k"] = prep["conv_k"][sl]
            m["conv_v"] = prep["conv_v"][sl]
        elif not host_conv:
            m["xkv"] = prep["xkv"][sl]
        maps.append(m)
    return maps


def kernel(**inputs) -> np.ndarray:
    global last_results
    x = np.asarray(inputs["x"], np.float32)
    B = x.shape[0]
    assert B == B_TOTAL, f"kernel hardcoded for B={B_TOTAL}, got {B}"

    prep = _host_prep(inputs)
    nc = _build_program()

    res = run_bass_kernel_spmd(nc, _in_maps(prep), core_ids=list(range(NCORES)))
    last_results = res

    # ---- gather + host combine ----
    ctxu = np.concatenate([r["ctxu"] for r in res.results])        # [B,12,65,1024]
    ecls = np.concatenate([r["ecls"] for r in res.results])        # [B,12,1024]
    kto = np.concatenate([r["kt_out"] for r in res.results])       # [B,128,6,256]
    vo = np.concatenate([r["v_out"] for r in res.results])         # [B,128,2,768]

    # K_conv [B, 256, 768]: ktile[p, et, j] holds KT[et*128+p, j]
    k_conv = kto.transpose(0, 3, 2, 1).reshape(B, NKV, EMBED)
    # V rows [B, 256, 768] from vaug[p, jt, h*(D)+d] = V[jt*128+p, h*64+d]
    v5 = vo.reshape(B, 128, 2, HEADS, D)                           # [B,128,2,12,64]
    v_conv = v5.transpose(0, 2, 1, 3, 4).reshape(B, NKV, EMBED)
    b_v = prep["b_v"]                                              # [768]
    v_conv = v_conv + b_v                                          # device omits b_v

    v_cls = prep["v_cls"]                                          # [B, 768]
    vch = v_cls.reshape(B, HEADS, D)
    bvh = b_v.reshape(HEADS, D)
    den = ctxu[:, :, D, :] + ecls                                  # [B,12,1024]
    num = (ctxu[:, :, :D, :]
           + ctxu[:, :, D:D + 1, :] * bvh[None, :, :, None]
           + ecls[:, :, None, :] * vch[:, :, :, None])
    ctx_pix = num / den[:, :, None, :]                             # [B,12,64,1024]
    out = np.empty((B, 1 + NPIX, EMBED), np.float32)
    out[:, 1:, :] = ctx_pix.transpose(0, 3, 1, 2).reshape(B, NPIX, EMBED)

    # cls-query row on host (exact fp32)
    k_all = np.concatenate([prep["k_cls"][:, None, :], k_conv], axis=1)  # [B,257,768]
    v_all = np.concatenate([v_cls[:, None, :], v_conv], axis=1)          # [B,257,768]
    qc = prep["q_cls"].reshape(B, HEADS, D)                              # [B,12,64]
    kh = k_all.reshape(B, 257, HEADS, D)
    vh = v_all.reshape(B, 257, HEADS, D)
    s = np.einsum("bhd,bjhd->bhj", qc, kh) * SM_SCALE
    s -= s.max(axis=2, keepdims=True)
    e = np.exp(s)
    p = e / e.sum(axis=2, keepdims=True)
    ctx0 = np.einsum("bhj,bjhd->bhd", p, vh)                             # [B,12,64]
    out[:, 0, :] = ctx0.reshape(B, EMBED)
    return out


# revision 37
# speedup vs baseline: 6.2058x; 6.2058x over previous
"""CvtSelfAttention TRN2 Bass kernel.

Strategy (8 NeuronCores, batch data-parallel, 4 batch elems per core):

The PE-column floor of this problem (fp32r/bf16 matmuls at 1 col/cycle) is
~110.6k columns per batch elem for projections + attention; the depthwise
convs would add another ~83k columns as diagonal matmuls, so they are moved
OFF the PE onto the DVE and Pool engines as bf16 multiply-add pair chains
(tensor_scalar -> tmp at 4x DVE rate, tensor_tensor add in place), leaving
the PE as the sole ~45us/batch bottleneck.

Device (per batch elem):
  1. Depthwise 3x3 convs in bf16 on DVE (q + v[ct<3]) and Pool (k + v[ct>=3]):
     9 taps, first tap fused with the BN shift via tensor_scalar, remaining
     8 taps as (ts mul -> tmp, tt add) pairs. Stride-2 k/v taps read an
     even/odd-plane repack so all views are contiguous (4x DVE mode).
     Emission is software-pipelined: conv(b+1) is emitted BEFORE the
     attention block of batch b so the DVE/Pool FIFOs never head-of-line
     block the next batch's conv behind attention evictions.
  2. QKV linear projections as bf16 matmuls (conv outputs x bf16 W^T tiles),
     fp32 PSUM, evicted with the torch-Linear bias by ACT (q/k) into f32r
     tiles, or scatter-copied by Pool into the V_aug layout (v; bias b_v is
     folded on the host into the final merge: + b_v * den).
  3. Per-head attention in f32r exactly as the baseline: scoresT = K^T Q
     (head pairs on PE row groups 0/64), exp on ACT (scores ~N(0,.1), no max
     subtraction), ctxT_aug = V_aug^T @ expT with a ones column giving the
     softmax denominator row. kproj/vproj of batch b+1 are interleaved as PE
     filler between heads: the ACT exp stream (2.1us/head) is slower than the
     PE scores+ctx stream (1.7us/head), and the filler absorbs the gap.
  4. cls-KEY scores for all 12 heads in one stuffed block-diagonal matmul.
Host (numpy, exact fp32; ~0.1% of FLOPs): input layout prep (bf16 padded /
even-odd planes, BN fold, W^T tiles), cls-query attention row, merge of the
cls-key term and b_v into ctx, final [B, L, E] assembly.
"""
import os
import sys
import numpy as np
import ml_dtypes

for _p in ("/opt/trn_rl_repo", "/root/.axon_site/_ro/trn_rl_repo"):
    if os.path.isdir(_p) and _p not in sys.path:
        sys.path.append(_p)

import concourse.bass as bass
import concourse.bacc as bacc
import concourse.tile as tile
from concourse import mybir
from concourse.bass_utils import run_bass_kernel_spmd

EMBED = 768
HEADS = 12
D = 64
EPS = 1e-5
NCORES = 8
B_TOTAL = 32
NB = B_TOTAL // NCORES          # batch elems per core
CT = EMBED // 128               # 6 c-tiles
NPIX = 1024                     # stride-1 conv output pixels (i-dim on device)
NKV = 256                       # stride-2 conv output pixels (j-dim on device)
SM_SCALE = float(EMBED) ** -0.5

F32 = mybir.dt.float32
F32R = mybir.dt.float32r
BF16 = mybir.dt.bfloat16
AO = mybir.AluOpType
BF = ml_dtypes.bfloat16

# set by kernel() for test harnesses to inspect
last_results = None


def _build_program(repeat: int = 1):
    nc = bacc.Bacc(None, target_bir_lowering=False, debug=False)

    host_conv = bool(os.environ.get("CONV_HOST"))
    kv_host = bool(os.environ.get("CONV_KV_HOST"))
    # ---- DRAM I/O (per core) ----
    if host_conv:
        cq_d = nc.dram_tensor("conv_q", [NB, 128, CT, NPIX], BF16,
                              kind="ExternalInput")
    else:
        xq_d = nc.dram_tensor("xq", [NB, CT, 128, 1156], BF16,
                              kind="ExternalInput")
    if host_conv or kv_host:
        ck_d = nc.dram_tensor("conv_k", [NB, 128, CT, NKV], BF16,
                              kind="ExternalInput")
        cv_d = nc.dram_tensor("conv_v", [NB, 128, CT, NKV], BF16,
                              kind="ExternalInput")
    elif not host_conv:
        xkv_d = nc.dram_tensor("xkv", [NB, CT, 128, 1156], BF16,
                               kind="ExternalInput")
    wcol = nc.dram_tensor("wcol", [128, 3 * CT * 9], F32, kind="ExternalInput")
    w_t = nc.dram_tensor("w_t", [3, CT, 128, EMBED], BF16, kind="ExternalInput")
    shifts = nc.dram_tensor("shifts", [128, 3 * CT], F32, kind="ExternalInput")
    biases = nc.dram_tensor("biases", [128, 2 * CT], F32, kind="ExternalInput")
    vtpl = nc.dram_tensor("vtpl", [128, HEADS * 128], F32R, kind="ExternalInput")
    kcls = nc.dram_tensor("kcls", [NB, 128, CT * HEADS], F32R, kind="ExternalInput")

    dbg_conv = None
    if os.environ.get("DBG_CONV"):
        dbg_conv = nc.dram_tensor("dbg_conv", [NB, 128, CT, NPIX + 2 * NKV], BF16,
                                  kind="ExternalOutput")
    ctxu = nc.dram_tensor("ctxu", [NB, HEADS, D + 1, NPIX], F32, kind="ExternalOutput")
    ecls = nc.dram_tensor("ecls", [NB, HEADS, NPIX], F32, kind="ExternalOutput")
    kt_out = nc.dram_tensor("kt_out", [NB, 128, CT, NKV], F32R, kind="ExternalOutput")
    v_out = nc.dram_tensor("v_out", [NB, 128, 2, HEADS * D], F32R,
                           kind="ExternalOutput")

    with tile.TileContext(nc) as tc:
        import contextlib
        with contextlib.ExitStack() as ctx:
            def _bufs(name, dflt):
                return int(os.environ.get(f"BUFS_{name}", dflt))

            consts = ctx.enter_context(tc.tile_pool(name="consts", bufs=1))
            xqp = ctx.enter_context(tc.tile_pool(name="xqp", bufs=_bufs("XQ", 3)))
            xkp = ctx.enter_context(tc.tile_pool(name="xkp", bufs=_bufs("XK", 3)))
            convp = ctx.enter_context(tc.tile_pool(name="convp", bufs=_bufs("CONV", 2)))
            tmpp = ctx.enter_context(tc.tile_pool(name="tmpp", bufs=2))
            qtp = ctx.enter_context(tc.tile_pool(name="qtp", bufs=_bufs("QT", 1)))
            ktp = ctx.enter_context(tc.tile_pool(name="ktp", bufs=_bufs("KT", 2)))
            vap = ctx.enter_context(tc.tile_pool(name="vap", bufs=_bufs("VA", 2)))
            kclsp = ctx.enter_context(tc.tile_pool(name="kclsp", bufs=2))
            expp = ctx.enter_context(tc.tile_pool(name="expp", bufs=_bufs("EXP", 3)))
            stage = ctx.enter_context(tc.tile_pool(name="stage", bufs=_bufs("ST", 3)))
            eclsp = ctx.enter_context(tc.tile_pool(name="eclsp", bufs=2))
            bigps = ctx.enter_context(
                tc.tile_pool(name="bigps", bufs=_bufs("BIG", 2), space="PSUM"))
            ps10 = ctx.enter_context(
                tc.tile_pool(name="ps10", bufs=_bufs("PS10", 2), space="PSUM"))

            # ---- constants ----
            if not host_conv:
                wcol_sb = consts.tile([128, 3 * CT * 9], F32)
                nc.sync.dma_start(wcol_sb, wcol[:, :])
                wcol_v = wcol_sb.rearrange("p (c t k) -> p c t k", c=3, t=CT)
                shifts_sb = consts.tile([128, 3 * CT], F32)
                nc.sync.dma_start(shifts_sb, shifts[:, :])
                shifts_v = shifts_sb.rearrange("p (c t) -> p c t", c=3)
            bias_sb = consts.tile([128, 2 * CT], F32)
            nc.sync.dma_start(bias_sb, biases[:, :])
            bias_v = bias_sb.rearrange("p (c t) -> p c t", c=2)
            ws = []
            for cv in range(3):
                row = [consts.tile([128, EMBED], BF16, name=f"w{cv}_{i}")
                       for i in range(CT)]
                for kt in range(CT):
                    nc.sync.dma_start(row[kt], w_t[cv, kt])
                ws.append(row)
            wq, wk, wv = ws
            # persistent V_aug double buffer: ones column (softmax denominator
            # row) written once; per-batch vproj evicts only touch cols 0:D of
            # each head block, so the template survives reuse.
            vaugs = [vap.tile([128, 2, HEADS * 128], F32R, tag="vaug",
                              name=f"vaug{i}") for i in range(2)]
            for vg in vaugs:
                nc.sync.dma_start(vg[:, 0, :], vtpl[:, :])
                nc.sync.dma_start(vg[:, 1, :], vtpl[:, :])

            def conv_chain(eng, dst, w_sc, shift_sc, view, nfree):
                # dst += sum_t w_t * x_t  (+shift on tap 0), all bf16
                tmp = tmpp.tile([128, nfree], BF16,
                                tag=f"tmp_{eng is nc.vector}_{nfree}")
                tv = tmp.rearrange("p (a b) -> p a b", b=view(0).shape[-1])
                eng.tensor_scalar(out=dst, in0=view(0), scalar1=w_sc(0),
                                  scalar2=shift_sc, op0=AO.mult, op1=AO.add)
                for tap in range(1, 9):
                    eng.tensor_scalar_mul(tv, view(tap), w_sc(tap))
                    eng.tensor_tensor(dst, tv, dst, AO.add)

            def emit_conv(b):
                """conv for batch b: host-computed DMA load, or DVE/Pool
                bf16 pair chains."""
                convq = convp.tile([128, CT, NPIX], BF16, tag="convq")
                convk = convp.tile([128, CT, NKV], BF16, tag="convk")
                convv = convp.tile([128, CT, NKV], BF16, tag="convv")
                if host_conv:
                    nc.sync.dma_start(convq, cq_d[b])
                if host_conv or kv_host:
                    nc.sync.dma_start(convk, ck_d[b])
                    nc.sync.dma_start(convv, cv_d[b])
                    if host_conv:
                        return convq, convk, convv
                for ct in range(CT):
                    xqt = xqp.tile([128, 1164], BF16, tag="xq")
                    nc.sync.dma_start(xqt[:, 0:1156], xq_d[b, ct])
                    if not kv_host:
                        xkt = xkp.tile([128, 1164], BF16, tag="xk")
                        nc.sync.dma_start(xkt[:, 0:1156], xkv_d[b, ct])

                    def qview(tap):
                        dh, dw = tap // 3, tap % 3
                        base = dh * 34 + dw
                        return xqt[:, base:base + 32 * 34].rearrange(
                            "p (r c) -> p r c", c=34)[:, :, 0:32]

                    def kview(tap):
                        dh, dw = tap // 3, tap % 3
                        base = (dw & 1) * 34 * 17 + dh * 17 + dw // 2
                        return xkt[:, base:base + 16 * 34].rearrange(
                            "p (r c) -> p r c", c=34)[:, :, 0:16]

                    conv_chain(
                        nc.vector,
                        convq[:, ct, :].rearrange("p (a b) -> p a b", b=32),
                        lambda t: wcol_v[:, 0, ct, t:t + 1],
                        shifts_v[:, 0, ct:ct + 1], qview, NPIX)
                    if not kv_host:
                        kv_eng = (nc.vector if os.environ.get("NO_POOL")
                                  else nc.gpsimd)
                        conv_chain(
                            kv_eng,
                            convk[:, ct, :].rearrange("p (a b) -> p a b", b=16),
                            lambda t: wcol_v[:, 1, ct, t:t + 1],
                            shifts_v[:, 1, ct:ct + 1], kview, NKV)
                        conv_chain(
                            kv_eng,
                            convv[:, ct, :].rearrange("p (a b) -> p a b", b=16),
                            lambda t: wcol_v[:, 2, ct, t:t + 1],
                            shifts_v[:, 2, ct:ct + 1], kview, NKV)
                return convq, convk, convv

            def emit_kproj_chunk(convk, ktile, et):
                pk = bigps.tile([128, NPIX], F32, tag="big")
                for kt in range(CT):
                    nc.tensor.matmul(
                        pk[:, 0:NKV], wk[kt][:, et * 128:(et + 1) * 128],
                        convk[:, kt, :],
                        start=(kt == 0), stop=(kt == CT - 1))
                nc.scalar.activation(
                    ktile[:, et, :], pk[:, 0:NKV],
                    mybir.ActivationFunctionType.Identity,
                    bias=bias_v[:, 1, et:et + 1])

            def emit_vproj_chunk(convv, vaug, jt):
                pv = bigps.tile([128, NPIX], F32, tag="big")
                for ch, (e0, en) in enumerate([(0, 512), (512, 256)]):
                    for kt in range(CT):
                        nc.tensor.matmul(
                            pv[:, e0:e0 + en],
                            convv[:, kt, jt * 128:(jt + 1) * 128],
                            wv[kt][:, e0:e0 + en],
                            start=(kt == 0), stop=(kt == CT - 1))
                # scatter heads into the 128-strided V_aug layout (ACT; Pool
                # cannot read PSUM)
                dstv = vaug[:, jt, :].rearrange("p (h d) -> p h d", d=128)[:, :, 0:D]
                nc.vector.tensor_copy(
                    dstv, pv[:, 0:EMBED].rearrange("p (h d) -> p h d", d=D))

            # =================== program ===================
            no_pipe = bool(os.environ.get("NO_PIPELINE"))
            # prologue: conv for batch 0
            conv_next = None if no_pipe else emit_conv(0)
            kv_cur = None          # (ktile, vaug) produced by fillers for b

            rep_ctx = tc.For_i(0, repeat, 1) if repeat > 1 else None
            if rep_ctx is not None:
                rep_ctx.__enter__()
            for b in range(NB):
                if no_pipe:
                    convq, convk, convv = emit_conv(b)
                else:
                    convq, convk, convv = conv_next
                if dbg_conv is not None:
                    nc.sync.dma_start(dbg_conv[b][:, :, 0:NPIX], convq)
                    nc.sync.dma_start(dbg_conv[b][:, :, NPIX:NPIX + NKV], convk)
                    nc.sync.dma_start(dbg_conv[b][:, :, NPIX + NKV:], convv)
                if not no_pipe:
                    # conv for next batch first: keeps DVE/Pool FIFOs ahead
                    conv_next = emit_conv((b + 1) % NB)
                if os.environ.get("CONV_ONLY"):
                    continue

                # ---- k/v projections for THIS batch if not already done
                # (batch 0 of each repeat iteration reuses the fillers from
                # the previous iteration's last batch; first time they must
                # be emitted inline) ----
                if kv_cur is None:
                    ktile = ktp.tile([128, CT, NKV], F32R, tag="ktile")
                    vaug = vaugs[b % 2]
                    for et in range(CT):
                        emit_kproj_chunk(convk, ktile, et)
                    for jt in range(2):
                        emit_vproj_chunk(convv, vaug, jt)
                    nc.sync.dma_start(kt_out[b], ktile[:, :, :])
                    nc.sync.dma_start(
                        v_out[b],
                        vaug.rearrange("p j (h d) -> p j h d", d=128)[:, :, :, 0:D])
                else:
                    ktile, vaug = kv_cur

                # ---- q projection ----
                qt = qtp.tile([128, CT, NPIX], F32R, tag="qt")
                for et in range(CT):
                    pq = bigps.tile([128, NPIX], F32, tag="big")
                    for ch in range(2):
                        for kt in range(CT):
                            nc.tensor.matmul(
                                pq[:, ch * 512:(ch + 1) * 512],
                                wq[kt][:, et * 128:(et + 1) * 128],
                                convq[:, kt, ch * 512:(ch + 1) * 512],
                                start=(kt == 0), stop=(kt == CT - 1))
                    nc.scalar.activation(
                        qt[:, et, :], pq, mybir.ActivationFunctionType.Identity,
                        bias=bias_v[:, 0, et:et + 1])

                # ---- cls-key scores for all heads ----
                kc = kclsp.tile([128, CT * HEADS], F32R)
                nc.sync.dma_start(kc, kcls[b])
                kc_v = kc.rearrange("p (t h) -> p t h", t=CT)
                pcls = bigps.tile([128, NPIX], F32, tag="big")
                for ch in range(2):
                    for kt in range(CT):
                        nc.tensor.matmul(
                            pcls[0:12, ch * 512:(ch + 1) * 512], kc_v[:, kt, :],
                            qt[:, kt, ch * 512:(ch + 1) * 512],
                            start=(kt == 0), stop=(kt == CT - 1))
                ec = eclsp.tile([12, NPIX], F32)
                nc.scalar.activation(ec, pcls[0:12, :],
                                     mybir.ActivationFunctionType.Exp,
                                     scale=SM_SCALE)
                nc.sync.dma_start(ecls[b], ec)

                # ---- k/v projections for NEXT batch, as attention fillers.
                # Batch 0 of each repeat iteration is computed inline instead
                # (its reads are trace-bound to the inline tiles), so the last
                # batch runs without fillers. ----
                bn = (b + 1) % NB
                fillers = []
                if bn != 0 and not no_pipe and not os.environ.get("NO_FILLERS"):
                    nktile = ktp.tile([128, CT, NKV], F32R, tag="ktile")
                    nvaug = vaugs[bn % 2]
                    nconvk, nconvv = conv_next[1], conv_next[2]
                    for et in range(CT):
                        fillers.append(
                            lambda et=et: emit_kproj_chunk(nconvk, nktile, et))
                    for jt in range(2):
                        fillers.append(
                            lambda jt=jt: emit_vproj_chunk(nconvv, nvaug, jt))
                    fillers.append(
                        lambda: nc.sync.dma_start(kt_out[bn], nktile[:, :, :]))
                    fillers.append(lambda: nc.sync.dma_start(
                        v_out[bn],
                        nvaug.rearrange("p j (h d) -> p j h d", d=128)[:, :, :, 0:D]))
                    kv_cur = (nktile, nvaug)
                else:
                    kv_cur = None
                fit = iter(fillers)

                # ---- attention ----
                for g in range(CT):  # head pairs on PE row groups 0/64
                    for hh in range(2):
                        h = 2 * g + hh
                        p0, p1 = 64 * hh, 64 * hh + 64
                        et = [None, None]
                        for jt in range(2):
                            pst = ps10.tile([128, NPIX], F32, tag="ps10")
                            for ch in range(2):
                                nc.tensor.matmul(
                                    pst[:, ch * 512:(ch + 1) * 512],
                                    ktile[p0:p1, g, jt * 128:(jt + 1) * 128],
                                    qt[p0:p1, g, ch * 512:(ch + 1) * 512],
                                    start=True, stop=True)
                            ex = expp.tile([128, NPIX], F32R)
                            nc.scalar.activation(
                                ex, pst, mybir.ActivationFunctionType.Exp,
                                scale=SM_SCALE)
                            et[jt] = ex
                        for fn in (next(fit, None),):
                            if fn is not None:
                                fn()
                        pc = bigps.tile([128, NPIX], F32, tag="big")
                        for ch in range(2):
                            for jt in range(2):
                                nc.tensor.matmul(
                                    pc[:, ch * 512:(ch + 1) * 512],
                                    vaug[:, jt, h * 128:(h + 1) * 128],
                                    et[jt][:, ch * 512:(ch + 1) * 512],
                                    start=(jt == 0), stop=(jt == 1))
                        st = stage.tile([D + 1, NPIX], F32)
                        nc.vector.tensor_copy(st, pc[0:D + 1, :])
                        nc.sync.dma_start(ctxu[b, h], st)
                # drain unused fillers (12 heads, 12 fillers -> none left)
                for fn in fit:
                    fn()
            if rep_ctx is not None:
                rep_ctx.__exit__(None, None, None)

    nc.finalize()
    return nc


def _host_prep(inputs):
    x = np.ascontiguousarray(inputs["x"], dtype=np.float32)     # [B, 1025, 768]
    B = x.shape[0]
    prep = {}

    cw = {}
    shift = {}
    for p in ["q", "k", "v"]:
        g = np.asarray(inputs[f"bn_g_{p}"], np.float32)
        be = np.asarray(inputs[f"bn_b_{p}"], np.float32)
        m = np.asarray(inputs[f"bn_m_{p}"], np.float32)
        v = np.asarray(inputs[f"bn_v_{p}"], np.float32)
        s = g / np.sqrt(v + EPS)
        cw[p] = np.asarray(inputs[f"conv_w_{p}"], np.float32)[:, 0] * s[:, None, None]
        shift[p] = be - m * s

    # padded 34x34 CHW image, bf16
    hs = x[:, 1:, :].reshape(B, 32, 32, EMBED)
    xp = np.zeros((B, EMBED, 34, 34), np.float32)
    xp[:, :, 1:33, 1:33] = hs.transpose(0, 3, 1, 2)
    host_conv = bool(os.environ.get("CONV_HOST"))
    kv_host = bool(os.environ.get("CONV_KV_HOST"))
    host_projs = ([("conv_q", "q", 1), ("conv_k", "k", 2), ("conv_v", "v", 2)]
                  if host_conv else
                  ([("conv_k", "k", 2), ("conv_v", "v", 2)] if kv_host else []))
    for name, p, stride in host_projs:
        n = 32 // stride
        acc = np.broadcast_to(shift[p][None, :, None, None],
                              (B, EMBED, n, n)).copy()
        for dh in range(3):
            for dw in range(3):
                acc += cw[p][:, dh, dw][None, :, None, None] * \
                    xp[:, :, dh:dh + 32:stride, dw:dw + 32:stride]
        prep[name] = np.ascontiguousarray(
            acc.reshape(B, CT, 128, n * n).transpose(0, 2, 1, 3).astype(BF))
    if not host_conv:
        xp_bf = xp.astype(BF)
        prep["xq"] = np.ascontiguousarray(xp_bf.reshape(B, CT, 128, 1156))
        if not kv_host:
            # even/odd column planes for stride-2 convs: [B,CT,128,2,34,17]
            xkv = np.empty((B, EMBED, 2, 34, 17), BF)
            xkv[:, :, 0] = xp_bf[:, :, :, 0::2]
            xkv[:, :, 1] = xp_bf[:, :, :, 1::2]
            prep["xkv"] = np.ascontiguousarray(xkv.reshape(B, CT, 128, 1156))

    # wcol: [128, 3*CT*9] per-partition conv-tap weights
    wc = np.zeros((128, 3, CT, 9), np.float32)
    for ci, p in enumerate(["q", "k", "v"]):
        w9 = cw[p].reshape(EMBED, 9)                  # [c, tap]
        for ct in range(CT):
            wc[:, ci, ct, :] = w9[ct * 128:(ct + 1) * 128, :]
    prep["wcol"] = np.ascontiguousarray(wc.reshape(128, -1))

    # w_t: [3, 6, 128, 768] bf16: W^T split into k-tiles
    prep["w_t"] = np.ascontiguousarray(np.stack([
        np.asarray(inputs[f"W_{p}"], np.float32).T.reshape(CT, 128, EMBED)
        for p in ["q", "k", "v"]]).astype(BF))

    # shifts [128, 3*6], biases [128, 2*6]
    sh = np.stack([shift[p].reshape(CT, 128) for p in ["q", "k", "v"]])  # [3,6,128]
    prep["shifts"] = np.ascontiguousarray(sh.transpose(2, 0, 1).reshape(128, -1))
    bb = np.stack([np.asarray(inputs[f"b_{p}"], np.float32).reshape(CT, 128)
                   for p in ["q", "k"]])                                  # [2,6,128]
    prep["biases"] = np.ascontiguousarray(bb.transpose(2, 0, 1).reshape(128, -1))

    # host-exact cls projections
    cls = x[:, 0, :]                                               # [B, 768]
    Wq = np.asarray(inputs["W_q"], np.float32)
    Wk = np.asarray(inputs["W_k"], np.float32)
    Wv = np.asarray(inputs["W_v"], np.float32)
    prep["q_cls"] = cls @ Wq.T + np.asarray(inputs["b_q"], np.float32)
    k_cls = cls @ Wk.T + np.asarray(inputs["b_k"], np.float32)
    prep["k_cls"] = k_cls
    prep["v_cls"] = cls @ Wv.T + np.asarray(inputs["b_v"], np.float32)
    prep["b_v"] = np.asarray(inputs["b_v"], np.float32)

    vt = np.zeros((128, HEADS, 128), np.float32)
    vt[:, :, D] = 1.0
    prep["vtpl"] = np.ascontiguousarray(vt.reshape(128, HEADS * 128))

    # kcls stuffed block lhsT: [B, 128, 6*12]
    kc = np.zeros((B, CT, 128, HEADS), np.float32)
    crange = np.arange(EMBED)
    hofc = crange // D                                             # head of channel
    for b in range(B):
        kc[b, crange // 128, crange % 128, hofc] = k_cls[b]
    prep["kcls"] = np.ascontiguousarray(kc.transpose(0, 2, 1, 3).reshape(B, 128, -1))
    return prep


def _in_maps(prep):
    maps = []
    for c in range(NCORES):
        sl = slice(c * NB, (c + 1) * NB)
        m = {
            "wcol": prep["wcol"],
            "w_t": prep["w_t"],
            "shifts": prep["shifts"],
            "biases": prep["biases"],
            "vtpl": prep["vtpl"],
            "kcls": prep["kcls"][sl],
        }
        host_conv = bool(os.environ.get("CONV_HOST"))
        kv_host = bool(os.environ.get("CONV_KV_HOST"))
        if host_conv:
            m["conv_q"] = prep["conv_q"][sl]
        else:
            m["xq"] = prep["xq"][sl]
        if host_conv or kv_host:
            m["conv_k"] = prep["conv_k"][sl]
            m["conv_v"] = prep["conv_v"][sl]
        elif not host_conv:
            m["xkv"] = prep["xkv"][sl]
        maps.append(m)
    return maps


def kernel(**inputs) -> np.ndarray:
    global last_results
    x = np.asarray(inputs["x"], np.float32)
    B = x.shape[0]
    assert B == B_TOTAL, f"kernel hardcoded for B={B_TOTAL}, got {B}"

    prep = _host_prep(inputs)
    nc = _build_program()

    res = run_bass_kernel_spmd(nc, _in_maps(prep), core_ids=list(range(NCORES)))
    last_results = res

    # ---- gather + host combine ----
    ctxu = np.concatenate([r["ctxu"] for r in res.results])        # [B,12,65,1024]
    ecls = np.concatenate([r["ecls"] for r in res.results])        # [B,12,1024]
    kto = np.concatenate([r["kt_out"] for r in res.results])       # [B,128,6,256]
    vo = np.concatenate([r["v_out"] for r in res.results])         # [B,128,2,768]

    # K_conv [B, 256, 768]: ktile[p, et, j] holds KT[et*128+p, j]
    k_conv = kto.transpose(0, 3, 2, 1).reshape(B, NKV, EMBED)
    # V rows [B, 256, 768] from vaug[p, jt, h*(D)+d] = V[jt*128+p, h*64+d]
    v5 = vo.reshape(B, 128, 2, HEADS, D)                           # [B,128,2,12,64]
    v_conv = v5.transpose(0, 2, 1, 3, 4).reshape(B, NKV, EMBED)
    b_v = prep["b_v"]                                              # [768]
    v_conv = v_conv + b_v                                          # device omits b_v

    v_cls = prep["v_cls"]                                          # [B, 768]
    vch = v_cls.reshape(B, HEADS, D)
    bvh = b_v.reshape(HEADS, D)
    den = ctxu[:, :, D, :] + ecls                                  # [B,12,1024]
    num = (ctxu[:, :, :D, :]
           + ctxu[:, :, D:D + 1, :] * bvh[None, :, :, None]
           + ecls[:, :, None, :] * vch[:, :, :, None])
    ctx_pix = num / den[:, :, None, :]                             # [B,12,64,1024]
    out = np.empty((B, 1 + NPIX, EMBED), np.float32)
    out[:, 1:, :] = ctx_pix.transpose(0, 3, 1, 2).reshape(B, NPIX, EMBED)

    # cls-query row on host (exact fp32)
    k_all = np.concatenate([prep["k_cls"][:, None, :], k_conv], axis=1)  # [B,257,768]
    v_all = np.concatenate([v_cls[:, None, :], v_conv], axis=1)          # [B,257,768]
    qc = prep["q_cls"].reshape(B, HEADS, D)                              # [B,12,64]
    kh = k_all.reshape(B, 257, HEADS, D)
    vh = v_all.reshape(B, 257, HEADS, D)
    s = np.einsum("bhd,bjhd->bhj", qc, kh) * SM_SCALE
    s -= s.max(axis=2, keepdims=True)
    e = np.exp(s)
    p = e / e.sum(axis=2, keepdims=True)
    ctx0 = np.einsum("bhj,bjhd->bhd", p, vh)                             # [B,12,64]
    out[:, 0, :] = ctx0.reshape(B, EMBED)
    return out


# revision 48
# speedup vs baseline: 6.7721x; 1.0912x over previous
"""CvtSelfAttention TRN2 Bass kernel.

Strategy (8 NeuronCores, batch data-parallel, 4 batch elems per core):

The PE-column floor of this problem (fp32r/bf16 matmuls at 1 col/cycle) is
~110.6k columns per batch elem for projections + attention; the depthwise
convs would add another ~83k columns as diagonal matmuls, so they are moved
OFF the PE onto the DVE and Pool engines as bf16 multiply-add pair chains
(tensor_scalar -> tmp at 4x DVE rate, tensor_tensor add in place), leaving
the PE as the sole ~45us/batch bottleneck.

Device (per batch elem):
  1. Depthwise 3x3 convs in bf16 on DVE (q + v[ct<3]) and Pool (k + v[ct>=3]):
     9 taps, first tap fused with the BN shift via tensor_scalar, remaining
     8 taps as (ts mul -> tmp, tt add) pairs. Stride-2 k/v taps read an
     even/odd-plane repack so all views are contiguous (4x DVE mode).
     Emission is software-pipelined: conv(b+1) is emitted BEFORE the
     attention block of batch b so the DVE/Pool FIFOs never head-of-line
     block the next batch's conv behind attention evictions.
  2. QKV linear projections as bf16 matmuls (conv outputs x bf16 W^T tiles),
     fp32 PSUM, evicted with the torch-Linear bias by ACT (q/k) into f32r
     tiles, or scatter-copied by Pool into the V_aug layout (v; bias b_v is
     folded on the host into the final merge: + b_v * den).
  3. Per-head attention in f32r exactly as the baseline: scoresT = K^T Q
     (head pairs on PE row groups 0/64), exp on ACT (scores ~N(0,.1), no max
     subtraction), ctxT_aug = V_aug^T @ expT with a ones column giving the
     softmax denominator row. kproj/vproj of batch b+1 are interleaved as PE
     filler between heads: the ACT exp stream (2.1us/head) is slower than the
     PE scores+ctx stream (1.7us/head), and the filler absorbs the gap.
  4. cls-KEY scores for all 12 heads in one stuffed block-diagonal matmul.
Host (numpy, exact fp32; ~0.1% of FLOPs): input layout prep (bf16 padded /
even-odd planes, BN fold, W^T tiles), cls-query attention row, merge of the
cls-key term and b_v into ctx, final [B, L, E] assembly.
"""
import os
import sys
import numpy as np
import ml_dtypes

for _p in ("/opt/trn_rl_repo", "/root/.axon_site/_ro/trn_rl_repo"):
    if os.path.isdir(_p) and _p not in sys.path:
        sys.path.append(_p)

import concourse.bass as bass
import concourse.bacc as bacc
import concourse.tile as tile
from concourse import mybir
from concourse.bass_utils import run_bass_kernel_spmd

EMBED = 768
HEADS = 12
D = 64
EPS = 1e-5
NCORES = 8
B_TOTAL = 32
NB = B_TOTAL // NCORES          # batch elems per core
CT = EMBED // 128               # 6 c-tiles
NPIX = 1024                     # stride-1 conv output pixels (i-dim on device)
NKV = 256                       # stride-2 conv output pixels (j-dim on device)
SM_SCALE = float(EMBED) ** -0.5

F32 = mybir.dt.float32
F32R = mybir.dt.float32r
BF16 = mybir.dt.bfloat16
AO = mybir.AluOpType
BF = ml_dtypes.bfloat16

# set by kernel() for test harnesses to inspect
last_results = None


def _build_program(repeat: int = 1):
    nc = bacc.Bacc(None, target_bir_lowering=False, debug=False)

    host_conv = bool(os.environ.get("CONV_HOST"))
    kv_host = bool(os.environ.get("CONV_KV_HOST"))
    # ---- DRAM I/O (per core) ----
    if host_conv:
        cq_d = nc.dram_tensor("conv_q", [NB, 128, CT, NPIX], BF16,
                              kind="ExternalInput")
    else:
        xq_d = nc.dram_tensor("xq", [NB, CT, 128, 1156], BF16,
                              kind="ExternalInput")
    if host_conv or kv_host:
        ck_d = nc.dram_tensor("conv_k", [NB, 128, CT, NKV], BF16,
                              kind="ExternalInput")
        cv_d = nc.dram_tensor("conv_v", [NB, 128, CT, NKV], BF16,
                              kind="ExternalInput")
    elif not host_conv:
        xkv_d = nc.dram_tensor("xkv", [NB, CT, 128, 1156], BF16,
                               kind="ExternalInput")
    wcol = nc.dram_tensor("wcol", [128, 3 * CT * 9], F32, kind="ExternalInput")
    w_t = nc.dram_tensor("w_t", [3, CT, 128, EMBED], BF16, kind="ExternalInput")
    shifts = nc.dram_tensor("shifts", [128, 3 * CT], F32, kind="ExternalInput")
    biases = nc.dram_tensor("biases", [128, 2 * CT], F32, kind="ExternalInput")
    vtpl = nc.dram_tensor("vtpl", [128, HEADS * 128], F32R, kind="ExternalInput")
    kcls = nc.dram_tensor("kcls", [NB, 128, CT * HEADS], F32R, kind="ExternalInput")

    dbg_conv = None
    if os.environ.get("DBG_CONV"):
        dbg_conv = nc.dram_tensor("dbg_conv", [NB, 128, CT, NPIX + 2 * NKV], BF16,
                                  kind="ExternalOutput")
    ctxu = nc.dram_tensor("ctxu", [NB, HEADS, D + 1, NPIX], BF16,
                          kind="ExternalOutput")
    ecls = nc.dram_tensor("ecls", [NB, HEADS, NPIX], F32, kind="ExternalOutput")
    kt_out = nc.dram_tensor("kt_out", [NB, 128, CT, NKV], F32R, kind="ExternalOutput")
    v_out = nc.dram_tensor("v_out", [NB, 128, 2, HEADS * D], F32R,
                           kind="ExternalOutput")

    with tile.TileContext(nc) as tc:
        import contextlib
        with contextlib.ExitStack() as ctx:
            def _bufs(name, dflt):
                return int(os.environ.get(f"BUFS_{name}", dflt))

            consts = ctx.enter_context(tc.tile_pool(name="consts", bufs=1))
            xqp = ctx.enter_context(tc.tile_pool(name="xqp", bufs=_bufs("XQ", 3)))
            xkp = ctx.enter_context(tc.tile_pool(name="xkp", bufs=_bufs("XK", 3)))
            convp = ctx.enter_context(tc.tile_pool(name="convp", bufs=_bufs("CONV", 2)))
            tmpp = ctx.enter_context(tc.tile_pool(name="tmpp", bufs=2))
            qtp = ctx.enter_context(tc.tile_pool(name="qtp", bufs=_bufs("QT", 1)))
            ktp = ctx.enter_context(tc.tile_pool(name="ktp", bufs=_bufs("KT", 2)))
            vap = ctx.enter_context(tc.tile_pool(name="vap", bufs=_bufs("VA", 2)))
            kclsp = ctx.enter_context(tc.tile_pool(name="kclsp", bufs=2))
            expp = ctx.enter_context(tc.tile_pool(name="expp", bufs=_bufs("EXP", 4)))
            stage = ctx.enter_context(tc.tile_pool(name="stage", bufs=_bufs("ST", 3)))
            eclsp = ctx.enter_context(tc.tile_pool(name="eclsp", bufs=2))
            bigps = ctx.enter_context(
                tc.tile_pool(name="bigps", bufs=_bufs("BIG", 2), space="PSUM"))
            ps10 = ctx.enter_context(
                tc.tile_pool(name="ps10", bufs=_bufs("PS10", 2), space="PSUM"))

            # ---- constants ----
            if not host_conv:
                wcol_sb = consts.tile([128, 3 * CT * 9], F32)
                nc.sync.dma_start(wcol_sb, wcol[:, :])
                wcol_v = wcol_sb.rearrange("p (c t k) -> p c t k", c=3, t=CT)
                shifts_sb = consts.tile([128, 3 * CT], F32)
                nc.sync.dma_start(shifts_sb, shifts[:, :])
                shifts_v = shifts_sb.rearrange("p (c t) -> p c t", c=3)
            bias_sb = consts.tile([128, 2 * CT], F32)
            nc.sync.dma_start(bias_sb, biases[:, :])
            bias_v = bias_sb.rearrange("p (c t) -> p c t", c=2)
            ws = []
            for cv in range(3):
                row = [consts.tile([128, EMBED], BF16, name=f"w{cv}_{i}")
                       for i in range(CT)]
                for kt in range(CT):
                    nc.sync.dma_start(row[kt], w_t[cv, kt])
                ws.append(row)
            wq, wk, wv = ws
            # persistent V_aug double buffer: ones column (softmax denominator
            # row) written once; per-batch vproj evicts only touch cols 0:D of
            # each head block, so the template survives reuse.
            vaugs = [vap.tile([128, 2, HEADS * 128], F32R, tag="vaug",
                              name=f"vaug{i}") for i in range(2)]
            for vg in vaugs:
                nc.sync.dma_start(vg[:, 0, :], vtpl[:, :])
                nc.sync.dma_start(vg[:, 1, :], vtpl[:, :])

            def conv_chain(eng, dst, w_sc, shift_sc, view, nfree):
                # dst += sum_t w_t * x_t  (+shift on tap 0), all bf16
                tmp = tmpp.tile([128, nfree], BF16,
                                tag=f"tmp_{eng is nc.vector}_{nfree}")
                tv = tmp.rearrange("p (a b) -> p a b", b=view(0).shape[-1])
                eng.tensor_scalar(out=dst, in0=view(0), scalar1=w_sc(0),
                                  scalar2=shift_sc, op0=AO.mult, op1=AO.add)
                for tap in range(1, 9):
                    eng.tensor_scalar_mul(tv, view(tap), w_sc(tap))
                    eng.tensor_tensor(dst, tv, dst, AO.add)

            def emit_conv(b):
                """conv for batch b: host-computed DMA load, or DVE/Pool
                bf16 pair chains."""
                convq = convp.tile([128, CT, NPIX], BF16, tag="convq")
                convk = convp.tile([128, CT, NKV], BF16, tag="convk")
                convv = convp.tile([128, CT, NKV], BF16, tag="convv")
                if host_conv:
                    nc.sync.dma_start(convq, cq_d[b])
                if host_conv or kv_host:
                    nc.sync.dma_start(convk, ck_d[b])
                    nc.sync.dma_start(convv, cv_d[b])
                    if host_conv:
                        return convq, convk, convv
                for ct in range(CT):
                    xqt = xqp.tile([128, 1164], BF16, tag="xq")
                    nc.sync.dma_start(xqt[:, 0:1156], xq_d[b, ct])
                    if not kv_host:
                        xkt = xkp.tile([128, 1164], BF16, tag="xk")
                        nc.sync.dma_start(xkt[:, 0:1156], xkv_d[b, ct])

                    def qview(tap):
                        dh, dw = tap // 3, tap % 3
                        base = dh * 34 + dw
                        return xqt[:, base:base + 32 * 34].rearrange(
                            "p (r c) -> p r c", c=34)[:, :, 0:32]

                    def kview(tap):
                        dh, dw = tap // 3, tap % 3
                        base = (dw & 1) * 34 * 17 + dh * 17 + dw // 2
                        return xkt[:, base:base + 16 * 34].rearrange(
                            "p (r c) -> p r c", c=34)[:, :, 0:16]

                    conv_chain(
                        nc.vector,
                        convq[:, ct, :].rearrange("p (a b) -> p a b", b=32),
                        lambda t: wcol_v[:, 0, ct, t:t + 1],
                        shifts_v[:, 0, ct:ct + 1], qview, NPIX)
                    if not kv_host:
                        kv_eng = (nc.vector if os.environ.get("NO_POOL")
                                  else nc.gpsimd)
                        conv_chain(
                            kv_eng,
                            convk[:, ct, :].rearrange("p (a b) -> p a b", b=16),
                            lambda t: wcol_v[:, 1, ct, t:t + 1],
                            shifts_v[:, 1, ct:ct + 1], kview, NKV)
                        conv_chain(
                            kv_eng,
                            convv[:, ct, :].rearrange("p (a b) -> p a b", b=16),
                            lambda t: wcol_v[:, 2, ct, t:t + 1],
                            shifts_v[:, 2, ct:ct + 1], kview, NKV)
                return convq, convk, convv

            def emit_kproj_half(convk, ktile, half):
                # K projection WITHOUT the Linear bias: a per-query-constant
                # logit shift cancels in softmax (the host folds -b_k into the
                # cls-key column and adds +b_k to the DMA'd K rows). 4 (or 2)
                # e-tiles share one PSUM tile and one eviction copy.
                ets = range(4) if half == 0 else range(4, CT)
                pk = bigps.tile([128, NPIX], F32, tag="big")
                for i, et in enumerate(ets):
                    for kt in range(CT):
                        nc.tensor.matmul(
                            pk[:, i * NKV:(i + 1) * NKV],
                            wk[kt][:, et * 128:(et + 1) * 128], convk[:, kt, :],
                            start=(kt == 0), stop=(kt == CT - 1))
                n = len(ets) * NKV
                nc.vector.tensor_copy(
                    ktile[:, ets[0]:ets[0] + len(ets), :],
                    pk[:, 0:n].rearrange("p (a b) -> p a b", b=NKV))

            def emit_vproj_chunk(convv, vaug, jt):
                pv = bigps.tile([128, NPIX], F32, tag="big")
                for ch, (e0, en) in enumerate([(0, 512), (512, 256)]):
                    for kt in range(CT):
                        nc.tensor.matmul(
                            pv[:, e0:e0 + en],
                            convv[:, kt, jt * 128:(jt + 1) * 128],
                            wv[kt][:, e0:e0 + en],
                            start=(kt == 0), stop=(kt == CT - 1))
                # scatter heads into the 128-strided V_aug layout (ACT; Pool
                # cannot read PSUM)
                dstv = vaug[:, jt, :].rearrange("p (h d) -> p h d", d=128)[:, :, 0:D]
                nc.vector.tensor_copy(
                    dstv, pv[:, 0:EMBED].rearrange("p (h d) -> p h d", d=D))

            # =================== program ===================
            no_pipe = bool(os.environ.get("NO_PIPELINE"))
            # prologue: conv for batch 0
            conv_next = None if no_pipe else emit_conv(0)
            kv_cur = None          # (ktile, vaug) produced by fillers for b

            rep_ctx = tc.For_i(0, repeat, 1) if repeat > 1 else None
            if rep_ctx is not None:
                rep_ctx.__enter__()
            for b in range(NB):
                if no_pipe:
                    convq, convk, convv = emit_conv(b)
                else:
                    convq, convk, convv = conv_next
                if dbg_conv is not None:
                    nc.sync.dma_start(dbg_conv[b][:, :, 0:NPIX], convq)
                    nc.sync.dma_start(dbg_conv[b][:, :, NPIX:NPIX + NKV], convk)
                    nc.sync.dma_start(dbg_conv[b][:, :, NPIX + NKV:], convv)
                if not no_pipe:
                    # conv for next batch first: keeps DVE/Pool FIFOs ahead
                    conv_next = emit_conv((b + 1) % NB)
                if os.environ.get("CONV_ONLY"):
                    continue

                # ---- k/v projections for THIS batch if not already done
                # (batch 0 of each repeat iteration reuses the fillers from
                # the previous iteration's last batch; first time they must
                # be emitted inline) ----
                if kv_cur is None:
                    ktile = ktp.tile([128, CT, NKV], F32R, tag="ktile")
                    vaug = vaugs[b % 2]
                    for half in range(2):
                        emit_kproj_half(convk, ktile, half)
                    for jt in range(2):
                        emit_vproj_chunk(convv, vaug, jt)
                    nc.sync.dma_start(kt_out[b], ktile[:, :, :])
                    nc.sync.dma_start(
                        v_out[b],
                        vaug.rearrange("p j (h d) -> p j h d", d=128)[:, :, :, 0:D])
                else:
                    ktile, vaug = kv_cur

                # ---- q projection ----
                qt = qtp.tile([128, CT, NPIX], F32R, tag="qt")
                for et in range(CT):
                    pq = bigps.tile([128, NPIX], F32, tag="big")
                    for ch in range(2):
                        for kt in range(CT):
                            nc.tensor.matmul(
                                pq[:, ch * 512:(ch + 1) * 512],
                                wq[kt][:, et * 128:(et + 1) * 128],
                                convq[:, kt, ch * 512:(ch + 1) * 512],
                                start=(kt == 0), stop=(kt == CT - 1))
                    nc.vector.tensor_scalar_add(qt[:, et, :], pq,
                                                bias_v[:, 0, et:et + 1])

                # ---- cls-key scores for all heads ----
                kc = kclsp.tile([128, CT * HEADS], F32R)
                nc.sync.dma_start(kc, kcls[b])
                kc_v = kc.rearrange("p (t h) -> p t h", t=CT)
                pcls = bigps.tile([128, NPIX], F32, tag="big")
                for ch in range(2):
                    for kt in range(CT):
                        nc.tensor.matmul(
                            pcls[0:12, ch * 512:(ch + 1) * 512], kc_v[:, kt, :],
                            qt[:, kt, ch * 512:(ch + 1) * 512],
                            start=(kt == 0), stop=(kt == CT - 1))
                ec = eclsp.tile([12, NPIX], F32)
                nc.scalar.activation(ec, pcls[0:12, :],
                                     mybir.ActivationFunctionType.Exp,
                                     scale=SM_SCALE)
                nc.sync.dma_start(ecls[b], ec)

                # ---- k/v projections for NEXT batch, as attention fillers.
                # Batch 0 of each repeat iteration is computed inline instead
                # (its reads are trace-bound to the inline tiles), so the last
                # batch runs without fillers. ----
                bn = (b + 1) % NB
                fillers = []
                if bn != 0 and not no_pipe and not os.environ.get("NO_FILLERS"):
                    nktile = ktp.tile([128, CT, NKV], F32R, tag="ktile")
                    nvaug = vaugs[bn % 2]
                    nconvk, nconvv = conv_next[1], conv_next[2]
                    for half in range(2):
                        fillers.append(
                            lambda half=half: emit_kproj_half(nconvk, nktile, half))
                    for jt in range(2):
                        fillers.append(
                            lambda jt=jt: emit_vproj_chunk(nconvv, nvaug, jt))
                    fillers.append(
                        lambda: nc.sync.dma_start(kt_out[bn], nktile[:, :, :]))
                    fillers.append(lambda: nc.sync.dma_start(
                        v_out[bn],
                        nvaug.rearrange("p j (h d) -> p j h d", d=128)[:, :, :, 0:D]))
                    kv_cur = (nktile, nvaug)
                else:
                    kv_cur = None
                fit = iter(fillers)

                # ---- attention: ctx runs one head behind scores so the PE
                # never waits on the ACT exp stream; ctx outputs are staged
                # in bf16 head-pairs (one copy + one DMA per two heads) ----
                st2 = [None]

                def emit_ctx(h, ets2):
                    pc = bigps.tile([128, NPIX], F32, tag="big")
                    for ch in range(2):
                        for jt in range(2):
                            nc.tensor.matmul(
                                pc[:, ch * 512:(ch + 1) * 512],
                                vaug[:, jt, h * 128:(h + 1) * 128],
                                ets2[jt][:, ch * 512:(ch + 1) * 512],
                                start=(jt == 0), stop=(jt == 1))
                    if h % 2 == 0:
                        st2[0] = stage.tile([D + 1, 2 * NPIX], BF16, name="st2")
                        nc.vector.tensor_copy(st2[0][:, 0:NPIX], pc[0:D + 1, :])
                    else:
                        nc.vector.tensor_copy(st2[0][:, NPIX:], pc[0:D + 1, :])
                        nc.sync.dma_start(
                            ctxu[b, h - 1:h + 1].rearrange("h p n -> p h n"),
                            st2[0].rearrange("p (h n) -> p h n", h=2))

                prev = None
                for g in range(CT):  # head pairs on PE row groups 0/64
                    for hh in range(2):
                        h = 2 * g + hh
                        p0, p1 = 64 * hh, 64 * hh + 64
                        et = [None, None]
                        for jt in range(2):
                            pst = ps10.tile([128, NPIX], F32, tag="ps10")
                            for ch in range(2):
                                nc.tensor.matmul(
                                    pst[:, ch * 512:(ch + 1) * 512],
                                    ktile[p0:p1, g, jt * 128:(jt + 1) * 128],
                                    qt[p0:p1, g, ch * 512:(ch + 1) * 512],
                                    start=True, stop=True)
                            ex = expp.tile([128, NPIX], F32R)
                            nc.scalar.activation(
                                ex, pst, mybir.ActivationFunctionType.Exp,
                                scale=SM_SCALE)
                            et[jt] = ex
                        for fn in (next(fit, None),):
                            if fn is not None:
                                fn()
                        if prev is not None:
                            emit_ctx(*prev)
                        prev = (h, et)
                emit_ctx(*prev)
                # drain unused fillers
                for fn in fit:
                    fn()
            if rep_ctx is not None:
                rep_ctx.__exit__(None, None, None)

    nc.finalize()
    return nc


def _host_prep(inputs):
    x = np.ascontiguousarray(inputs["x"], dtype=np.float32)     # [B, 1025, 768]
    B = x.shape[0]
    prep = {}

    cw = {}
    shift = {}
    for p in ["q", "k", "v"]:
        g = np.asarray(inputs[f"bn_g_{p}"], np.float32)
        be = np.asarray(inputs[f"bn_b_{p}"], np.float32)
        m = np.asarray(inputs[f"bn_m_{p}"], np.float32)
        v = np.asarray(inputs[f"bn_v_{p}"], np.float32)
        s = g / np.sqrt(v + EPS)
        cw[p] = np.asarray(inputs[f"conv_w_{p}"], np.float32)[:, 0] * s[:, None, None]
        shift[p] = be - m * s

    # padded 34x34 CHW image, bf16
    hs = x[:, 1:, :].reshape(B, 32, 32, EMBED)
    xp = np.zeros((B, EMBED, 34, 34), np.float32)
    xp[:, :, 1:33, 1:33] = hs.transpose(0, 3, 1, 2)
    host_conv = bool(os.environ.get("CONV_HOST"))
    kv_host = bool(os.environ.get("CONV_KV_HOST"))
    host_projs = ([("conv_q", "q", 1), ("conv_k", "k", 2), ("conv_v", "v", 2)]
                  if host_conv else
                  ([("conv_k", "k", 2), ("conv_v", "v", 2)] if kv_host else []))
    for name, p, stride in host_projs:
        n = 32 // stride
        acc = np.broadcast_to(shift[p][None, :, None, None],
                              (B, EMBED, n, n)).copy()
        for dh in range(3):
            for dw in range(3):
                acc += cw[p][:, dh, dw][None, :, None, None] * \
                    xp[:, :, dh:dh + 32:stride, dw:dw + 32:stride]
        prep[name] = np.ascontiguousarray(
            acc.reshape(B, CT, 128, n * n).transpose(0, 2, 1, 3).astype(BF))
    if not host_conv:
        xp_bf = xp.astype(BF)
        prep["xq"] = np.ascontiguousarray(xp_bf.reshape(B, CT, 128, 1156))
        if not kv_host:
            # even/odd column planes for stride-2 convs: [B,CT,128,2,34,17]
            xkv = np.empty((B, EMBED, 2, 34, 17), BF)
            xkv[:, :, 0] = xp_bf[:, :, :, 0::2]
            xkv[:, :, 1] = xp_bf[:, :, :, 1::2]
            prep["xkv"] = np.ascontiguousarray(xkv.reshape(B, CT, 128, 1156))

    # wcol: [128, 3*CT*9] per-partition conv-tap weights
    wc = np.zeros((128, 3, CT, 9), np.float32)
    for ci, p in enumerate(["q", "k", "v"]):
        w9 = cw[p].reshape(EMBED, 9)                  # [c, tap]
        for ct in range(CT):
            wc[:, ci, ct, :] = w9[ct * 128:(ct + 1) * 128, :]
    prep["wcol"] = np.ascontiguousarray(wc.reshape(128, -1))

    # w_t: [3, 6, 128, 768] bf16: W^T split into k-tiles
    prep["w_t"] = np.ascontiguousarray(np.stack([
        np.asarray(inputs[f"W_{p}"], np.float32).T.reshape(CT, 128, EMBED)
        for p in ["q", "k", "v"]]).astype(BF))

    # shifts [128, 3*6], biases [128, 2*6]
    sh = np.stack([shift[p].reshape(CT, 128) for p in ["q", "k", "v"]])  # [3,6,128]
    prep["shifts"] = np.ascontiguousarray(sh.transpose(2, 0, 1).reshape(128, -1))
    bb = np.stack([np.asarray(inputs[f"b_{p}"], np.float32).reshape(CT, 128)
                   for p in ["q", "k"]])                                  # [2,6,128]
    prep["biases"] = np.ascontiguousarray(bb.transpose(2, 0, 1).reshape(128, -1))

    # host-exact cls projections
    cls = x[:, 0, :]                                               # [B, 768]
    Wq = np.asarray(inputs["W_q"], np.float32)
    Wk = np.asarray(inputs["W_k"], np.float32)
    Wv = np.asarray(inputs["W_v"], np.float32)
    prep["q_cls"] = cls @ Wq.T + np.asarray(inputs["b_q"], np.float32)
    k_cls = cls @ Wk.T + np.asarray(inputs["b_k"], np.float32)
    prep["k_cls"] = k_cls
    prep["v_cls"] = cls @ Wv.T + np.asarray(inputs["b_v"], np.float32)
    prep["b_v"] = np.asarray(inputs["b_v"], np.float32)

    vt = np.zeros((128, HEADS, 128), np.float32)
    vt[:, :, D] = 1.0
    prep["vtpl"] = np.ascontiguousarray(vt.reshape(128, HEADS * 128))

    # kcls stuffed block lhsT: [B, 128, 6*12]. The device K projection omits
    # b_k (a uniform per-query logit shift cancels in softmax), so the cls
    # column must be shifted by the same amount: stuff (k_cls - b_k).
    kc = np.zeros((B, CT, 128, HEADS), np.float32)
    crange = np.arange(EMBED)
    hofc = crange // D                                             # head of channel
    k_cls_dev = k_cls - np.asarray(inputs["b_k"], np.float32)
    for b in range(B):
        kc[b, crange // 128, crange % 128, hofc] = k_cls_dev[b]
    prep["kcls"] = np.ascontiguousarray(kc.transpose(0, 2, 1, 3).reshape(B, 128, -1))
    prep["b_k"] = np.asarray(inputs["b_k"], np.float32)
    return prep


def _in_maps(prep):
    maps = []
    for c in range(NCORES):
        sl = slice(c * NB, (c + 1) * NB)
        m = {
            "wcol": prep["wcol"],
            "w_t": prep["w_t"],
            "shifts": prep["shifts"],
            "biases": prep["biases"],
            "vtpl": prep["vtpl"],
            "kcls": prep["kcls"][sl],
        }
        host_conv = bool(os.environ.get("CONV_HOST"))
        kv_host = bool(os.environ.get("CONV_KV_HOST"))
        if host_conv:
            m["conv_q"] = prep["conv_q"][sl]
        else:
            m["xq"] = prep["xq"][sl]
        if host_conv or kv_host:
            m["conv_k"] = prep["conv_k"][sl]
            m["conv_v"] = prep["conv_v"][sl]
        elif not host_conv:
            m["xkv"] = prep["xkv"][sl]
        maps.append(m)
    return maps


def kernel(**inputs) -> np.ndarray:
    global last_results
    x = np.asarray(inputs["x"], np.float32)
    B = x.shape[0]
    assert B == B_TOTAL, f"kernel hardcoded for B={B_TOTAL}, got {B}"

    prep = _host_prep(inputs)
    nc = _build_program()

    res = run_bass_kernel_spmd(nc, _in_maps(prep), core_ids=list(range(NCORES)))
    last_results = res

    # ---- gather + host combine ----
    ctxu = np.concatenate([r["ctxu"] for r in res.results]).astype(np.float32)
    ecls = np.concatenate([r["ecls"] for r in res.results])        # [B,12,1024]
    kto = np.concatenate([r["kt_out"] for r in res.results])       # [B,128,6,256]
    vo = np.concatenate([r["v_out"] for r in res.results])         # [B,128,2,768]

    # K_conv [B, 256, 768]: ktile[p, et, j] holds KT[et*128+p, j]; the device
    # omitted b_k, add it back for the host cls-query row.
    k_conv = kto.transpose(0, 3, 2, 1).reshape(B, NKV, EMBED) + prep["b_k"]
    # V rows [B, 256, 768] from vaug[p, jt, h*(D)+d] = V[jt*128+p, h*64+d]
    v5 = vo.reshape(B, 128, 2, HEADS, D)                           # [B,128,2,12,64]
    v_conv = v5.transpose(0, 2, 1, 3, 4).reshape(B, NKV, EMBED)
    b_v = prep["b_v"]                                              # [768]
    v_conv = v_conv + b_v                                          # device omits b_v

    v_cls = prep["v_cls"]                                          # [B, 768]
    vch = v_cls.reshape(B, HEADS, D)
    bvh = b_v.reshape(HEADS, D)
    den = ctxu[:, :, D, :] + ecls                                  # [B,12,1024]
    num = (ctxu[:, :, :D, :]
           + ctxu[:, :, D:D + 1, :] * bvh[None, :, :, None]
           + ecls[:, :, None, :] * vch[:, :, :, None])
    ctx_pix = num / den[:, :, None, :]                             # [B,12,64,1024]
    out = np.empty((B, 1 + NPIX, EMBED), np.float32)
    out[:, 1:, :] = ctx_pix.transpose(0, 3, 1, 2).reshape(B, NPIX, EMBED)

    # cls-query row on host (exact fp32)
    k_all = np.concatenate([prep["k_cls"][:, None, :], k_conv], axis=1)  # [B,257,768]
    v_all = np.concatenate([v_cls[:, None, :], v_conv], axis=1)          # [B,257,768]
    qc = prep["q_cls"].reshape(B, HEADS, D)                              # [B,12,64]
    kh = k_all.reshape(B, 257, HEADS, D)
    vh = v_all.reshape(B, 257, HEADS, D)
    s = np.einsum("bhd,bjhd->bhj", qc, kh) * SM_SCALE
    s -= s.max(axis=2, keepdims=True)
    e = np.exp(s)
    p = e / e.sum(axis=2, keepdims=True)
    ctx0 = np.einsum("bhj,bjhd->bhd", p, vh)                             # [B,12,64]
    out[:, 0, :] = ctx0.reshape(B, EMBED)
    return out
